# revision 40
# baseline (speedup 1.0000x reference)
"""DisplaceChannel kernel for Trainium2 (8 NeuronCores, Bass/Tile).

out = depthwise3x3(displace(inp, round(offset)), gaussian(offset - round(offset)))

Strategy (v5):
- Data-parallel over batch: 32 batches -> 4 per core.
- Positions packed 4 per tile (4 pos x 4 batch x 8 chan = 128 partitions),
  grouped by EQUAL integer x-offset (sorted by y-offset inside a group so
  the row-band union stays tight).
- y-displacement folded into the input DMA row placement; rows copied
  FULL-WIDTH so each channel transfer is one contiguous Hv*256B chunk.
- x-displacement folded into the x-conv access-pattern offsets (uniform
  within a group). Data sits in unshifted "u" coords until the x-conv
  writes shifted output coords.
- Separable 3-tap y-conv then 3-tap x-conv, band/window-restricted.
  Each (group, stage) runs either on ScalarE+VectorE (per-partition-scalar
  mul + two fused scalar_tensor_tensor MACs) or on the TensorEngine
  (3 accumulating diagonal-stationary fp32 matmuls into PSUM + ScalarE
  copy back to SBUF). A greedy balancer splits stages across PE and DVE.
- Output HBM is pre-zeroed by the runtime; each position's nonzero row
  band is written full-width (contiguous), zeros in off-band columns.
"""
import os
import sys

import numpy as np

for _p in ("/opt/trn_rl_repo", "/root/.axon_site/_ro/trn_rl_repo"):
    if os.path.isdir(_p) and _p not in sys.path:
        sys.path.insert(0, _p)
        break

from contextlib import ExitStack

import concourse.bass as bass  # noqa: F401
import concourse.tile as tile
from concourse import bacc, mybir
from concourse.bass_utils import run_bass_kernel_spmd

H = 64
W = 64
B = 32
CHAN_PER_POS = 8
NUM_POS = 48
C = NUM_POS * CHAN_PER_POS
SIGMA = 0.5
NCORES = 8
BL = B // NCORES
POS_PER_GROUP = 4
F32 = mybir.dt.float32

# engine cost weights (ns per output column) for the stage balancer
_PE_NS_PER_COL = 5.0    # 3 x 4 cyc/row @ 2.4 GHz, measured
_PE_NS_PER_CHUNK = 1270.0  # 3 x (ldweights + matmul fixed), measured
_DVE_NS_PER_COL = 2.08  # 2 stt ops @ 0.96 GHz, measured
_DVE_NS_PER_STAGE = 310.0


def _pe_cost(cols):
    import math
    if cols < 2800:
        # narrow stages are per-matmul-overhead dominated on PE; keep on DVE
        return float("inf")
    return cols * _PE_NS_PER_COL + math.ceil(cols / 512) * _PE_NS_PER_CHUNK


def _dve_cost(cols):
    return cols * _DVE_NS_PER_COL + _DVE_NS_PER_STAGE

_cache = {}


def _geometry(offset):
    off_round = np.round(offset)  # round-half-even, matches jnp.round
    oxy = off_round.astype(np.int64)
    frac = (offset - off_round).astype(np.float32)

    coords = (np.arange(3, dtype=np.float32) - np.float32(1.0))
    dx = coords[None, :] + frac[:, 0:1]
    dy = coords[None, :] + frac[:, 1:2]
    inv = np.float32(1.0 / (2.0 * SIGMA * SIGMA))
    gx = np.exp(-(dx * dx) * inv).astype(np.float32)
    gy = np.exp(-(dy * dy) * inv).astype(np.float32)
    wx = gx / gx.sum(axis=1, keepdims=True)
    wy = gy / gy.sum(axis=1, keepdims=True)

    pos = {}
    for p in range(NUM_POS):
        ox, oy = int(oxy[p, 0]), int(oxy[p, 1])
        vy0, vy1 = max(0, oy), min(H, H + oy)
        vx0, vx1 = max(0, ox), min(W, W + ox)
        if vy1 <= vy0 or vx1 <= vx0:
            continue
        pos[p] = dict(
            p=p, ox=ox, oy=oy, vy0=vy0, vy1=vy1,
            sy0=vy0 - oy, sx0=vx0 - ox, wv=vx1 - vx0,
            by0=max(0, vy0 - 1), by1=min(H, vy1 + 1),
            bx0=max(0, vx0 - 1), bx1=min(W, vx1 + 1),
        )

    by_ox = {}
    for p, m in sorted(pos.items(), key=lambda kv: (kv[1]["ox"], kv[1]["oy"])):
        by_ox.setdefault(m["ox"], []).append(m)

    groups = []
    for ox in sorted(by_ox):
        mem = by_ox[ox]
        for i in range(0, len(mem), POS_PER_GROUP):
            members = mem[i:i + POS_PER_GROUP]
            gby0 = min(m["by0"] for m in members)
            gby1 = max(m["by1"] for m in members)
            sx0 = members[0]["sx0"]
            wv = members[0]["wv"]
            ud0 = max(0, sx0 - 2)
            ud1 = min(W, sx0 + wv + 2)
            groups.append(dict(
                members=members, ox=ox, gby0=gby0, gby1=gby1,
                sx0=sx0, wv=wv, ud0=ud0, ud1=ud1,
                bx0=members[0]["bx0"], bx1=members[0]["bx1"],
            ))

    # greedy PE/DVE balance over (group, stage) units
    units = []
    for g, grp in enumerate(groups):
        bg = grp["gby1"] - grp["gby0"]
        units.append((bg * (grp["ud1"] - grp["ud0"]), g, "y"))
        units.append((bg * (grp["bx1"] - grp["bx0"]), g, "x"))
    # walk in pipeline order so PE/DVE stages interleave in time
    units.sort(key=lambda u: (u[1], u[2] == "x"))
    pe_load, dve_load = 0.0, 0.0
    assign = {}
    use_pe = os.environ.get("KERNEL_USE_PE", "") == "1"
    for cols, g, st in units:
        if use_pe and max(pe_load + _pe_cost(cols), dve_load) <= \
           max(pe_load, dve_load + _dve_cost(cols)):
            assign[(g, st)] = "pe"
            pe_load += _pe_cost(cols)
        else:
            assign[(g, st)] = "dve"
            dve_load += _dve_cost(cols)
    for g, grp in enumerate(groups):
        grp["eng_y"] = assign[(g, "y")]
        grp["eng_x"] = assign[(g, "x")]

    ng = len(groups)
    taps = np.zeros((128, max(ng, 1) * 6), dtype=np.float32)
    for g, grp in enumerate(groups):
        for i, m in enumerate(grp["members"]):
            rows = slice(i * 32, (i + 1) * 32)
            for k in range(3):
                taps[rows, g * 6 + k] = wy[m["p"], k]
                taps[rows, g * 6 + 3 + k] = wx[m["p"], k]

    # diagonal stationaries for PE-assigned stages: [128, nd*128]
    diag_cols = []
    for g, grp in enumerate(groups):
        for st, wmat in (("y", wy), ("x", wx)):
            if grp["eng_" + st] != "pe":
                continue
            grp["diag_" + st] = len(diag_cols)
            for k in range(3):
                dcol = np.zeros((128, 128), dtype=np.float32)
                for i, m in enumerate(grp["members"]):
                    for q in range(i * 32, (i + 1) * 32):
                        dcol[q, q] = wmat[m["p"], k]
                diag_cols.append(dcol)
    diags = (np.concatenate(diag_cols, axis=1) if diag_cols
             else np.zeros((128, 128), dtype=np.float32))
    return groups, taps, diags


def _build(groups, n_tap_cols, n_diag_cols):
    nc = bacc.Bacc("TRN2", target_bir_lowering=False, debug=False,
                   num_devices=NCORES)
    inp_d = nc.dram_tensor("inp", [BL, C, H, W], F32, kind="ExternalInput")
    taps_d = nc.dram_tensor("taps", [128, n_tap_cols], F32, kind="ExternalInput")
    diags_d = nc.dram_tensor("diags", [128, n_diag_cols], F32,
                             kind="ExternalInput")
    out_d = nc.dram_tensor("out", [BL, C, H, W], F32, kind="ExternalOutput")

    mult = mybir.AluOpType.mult
    add = mybir.AluOpType.add
    dma_ctr = [0]

    with tile.TileContext(nc) as tc:
        with ExitStack() as ctx:
            use_pe_b = os.environ.get("KERNEL_USE_PE", "") == "1"
            dpool = ctx.enter_context(tc.tile_pool(name="dpool", bufs=4))
            tpool = ctx.enter_context(
                tc.tile_pool(name="tpool", bufs=3 if use_pe_b else 4))
            opool = ctx.enter_context(tc.tile_pool(name="opool", bufs=3))
            cpool = ctx.enter_context(tc.tile_pool(name="cpool", bufs=1))
            pspool = ctx.enter_context(
                tc.tile_pool(name="pspool", bufs=8, space="PSUM"))

            taps_t = cpool.tile([128, n_tap_cols], F32, tag="taps")
            nc.sync.dma_start(taps_t[:], taps_d.ap()[:, :])
            diags_t = cpool.tile([128, n_diag_cols], F32, tag="diags")
            nc.sync.dma_start(diags_t[:], diags_d.ap()[:, :])

            def tap(g, k):
                return taps_t[:, g * 6 + k:g * 6 + k + 1]

            def dma(dst, src):
                if os.environ.get("KERNEL_DMA_SYNC_ONLY", "") == "1":
                    eng = nc.sync
                else:
                    eng = (nc.sync, nc.scalar)[dma_ctr[0] % 2]
                dma_ctr[0] += 1
                eng.dma_start(dst, src)

            def stage_dve(out_ap, in_aps, tapbase, g):
                # split rows: the stt MACs on an early slice can start while
                # ScalarE is still doing a later slice's mul. Narrow stages
                # stay whole (per-op overhead dominates); wide ones split 2-4x
                rows = out_ap.shape[1]
                cols = rows * (out_ap.shape[2] if len(out_ap.shape) > 2 else 1)
                ns = max(1, min(4, cols // 1600))
                if os.environ.get("KERNEL_ROW_SPLIT", "1") != "1":
                    ns = 1
                step = -(-rows // ns)
                splits = tuple((r, min(rows, r + step))
                               for r in range(0, rows, step))
                for (ra, rb) in splits:
                    nc.scalar.mul(out_ap[:, ra:rb], in_aps[0][:, ra:rb],
                                  tap(g, tapbase))
                for (ra, rb) in splits:
                    nc.vector.scalar_tensor_tensor(
                        out_ap[:, ra:rb], in_aps[1][:, ra:rb],
                        tap(g, tapbase + 1), out_ap[:, ra:rb], mult, add)
                    nc.vector.scalar_tensor_tensor(
                        out_ap[:, ra:rb], in_aps[2][:, ra:rb],
                        tap(g, tapbase + 2), out_ap[:, ra:rb], mult, add)

            def stage_pe(out3, ocol0, in3, icol0, wcols, bg, rows_all,
                         diag_idx):
                # out3[:, r, ocol0:ocol0+wcols] =
                #   sum_k diag_k * in3[:, r+dr_k, icol0:icol0+wcols]
                rpc = max(1, 512 // wcols)
                nchunks = -(-bg // rpc)
                rpc = -(-bg // nchunks)  # equal-ish chunks, no tiny tail
                r = 0
                while r < bg:
                    nr = min(rpc, bg - r)
                    acc = pspool.tile([128, nr * wcols], F32, tag="ps")
                    accv = acc[:].rearrange("q (a b) -> q a b", b=wcols)
                    for k in range(3):
                        dr = k if rows_all else 0
                        dc = 0 if rows_all else k
                        nc.tensor.matmul(
                            acc[:, 0:nr * wcols],
                            diags_t[:, (diag_idx + k) * 128:
                                    (diag_idx + k + 1) * 128],
                            in3[:, r + dr:r + dr + nr,
                                icol0 + dc:icol0 + dc + wcols],
                            start=(k == 0), stop=(k == 2))
                    nc.scalar.copy(out3[:, r:r + nr, ocol0:ocol0 + wcols],
                                   accv[:, :, :])
                    r += nr

            for g, grp in enumerate(groups):
                gby0, gby1 = grp["gby0"], grp["gby1"]
                bg = gby1 - gby0
                drows = bg + 2
                ox = grp["ox"]
                sx0, wv, ud0, ud1 = grp["sx0"], grp["wv"], grp["ud0"], grp["ud1"]
                wd = ud1 - ud0
                wt = wd + 4
                bx0, bx1 = grp["bx0"], grp["bx1"]
                wb = bx1 - bx0

                d_t = dpool.tile([128, drows * W], F32, tag="D")
                d3 = d_t[:].rearrange("q (r c) -> q r c", c=W)
                nc.gpsimd.memset(d3[:, :, ud0:ud1], 0.0)

                # partition layout within a member: q = i*32 + ch*4 + b, so
                # the DMA can put the 8-value channel dim outermost (the SDMA
                # engine index follows the outermost AP dim -> 8 engines)
                for i, m in enumerate(grp["members"]):
                    hv = m["vy1"] - m["vy0"]
                    r0 = 1 + m["vy0"] - gby0
                    q0 = i * 32
                    dst = d_t[q0:q0 + 32, r0 * W:(r0 + hv) * W]
                    src = inp_d.ap()[:, 8 * m["p"]:8 * m["p"] + 8,
                                     m["sy0"]:m["sy0"] + hv, :]
                    dma(dst, src.rearrange("b ch r c -> ch b (r c)"))

                if sx0 > ud0:
                    nc.gpsimd.memset(d3[:, :, ud0:sx0], 0.0)
                if ud1 > sx0 + wv:
                    nc.gpsimd.memset(d3[:, :, sx0 + wv:ud1], 0.0)

                # y-conv: T[tr, 2+j] = sum_ky wy[ky] * D[tr+ky, ud0+j]
                t_t = tpool.tile([128, bg * wt], F32, tag="T")
                t3 = t_t[:].rearrange("q (r c) -> q r c", c=wt)
                nc.gpsimd.memset(t3[:, :, 0:2], 0.0)
                nc.gpsimd.memset(t3[:, :, wt - 2:wt], 0.0)
                if grp["eng_y"] == "pe":
                    stage_pe(t3, 2, d3, ud0, wd, bg, True, grp["diag_y"])
                else:
                    tdat = t3[:, :, 2:2 + wd]
                    stage_dve(tdat,
                              [d3[:, k:k + bg, ud0:ud1] for k in range(3)],
                              0, g)

                # x-conv: O[tr, x] = sum_kx wx[kx] * T[tr, x-ox+kx-1-ud0+2]
                o_t = opool.tile([128, bg * W], F32, tag="O")
                o3 = o_t[:].rearrange("q (r c) -> q r c", c=W)
                if bx0 > 0:
                    nc.gpsimd.memset(o3[:, :, 0:bx0], 0.0)
                if bx1 < W:
                    nc.gpsimd.memset(o3[:, :, bx1:W], 0.0)
                c0 = bx0 - ox - 1 - ud0 + 2
                if grp["eng_x"] == "pe":
                    stage_pe(o3, bx0, t3, c0, wb, bg, False, grp["diag_x"])
                else:
                    odat = o3[:, :, bx0:bx1]
                    stage_dve(odat,
                              [t3[:, :, c0 + k:c0 + k + wb] for k in range(3)],
                              3, g)

                for i, m in enumerate(grp["members"]):
                    r0, r1 = m["by0"] - gby0, m["by1"] - gby0
                    q0 = i * 32
                    src = o_t[q0:q0 + 32, r0 * W:r1 * W]
                    dst = out_d.ap()[:, 8 * m["p"]:8 * m["p"] + 8,
                                     m["by0"]:m["by1"], :]
                    dma(dst.rearrange("b ch r c -> ch b (r c)"), src)

    nc.compile()
    return nc


def kernel(inp, offset):
    inp = np.ascontiguousarray(inp, dtype=np.float32)
    offset = np.ascontiguousarray(offset, dtype=np.float32)
    assert inp.shape == (B, C, H, W), inp.shape

    key = offset.tobytes()
    if key not in _cache:
        groups, taps, diags = _geometry(offset)
        nc = _build(groups, taps.shape[1], diags.shape[1])
        _cache[key] = (nc, taps, diags)
    nc, taps, diags = _cache[key]

    in_maps = [{"inp": inp[c * BL:(c + 1) * BL], "taps": taps, "diags": diags}
               for c in range(NCORES)]
    trace = os.environ.get("KERNEL_TRACE", "") == "1"
    try:
        res = run_bass_kernel_spmd(nc, in_maps, core_ids=list(range(NCORES)),
                                   trace=trace)
    except ModuleNotFoundError:
        # NTFF profile hook unavailable; run untraced
        trace = False
        res = run_bass_kernel_spmd(nc, in_maps, core_ids=list(range(NCORES)),
                                   trace=False)
    if trace:
        print(f"HW exec time: {res.exec_time_ns} ns "
              f"(mean {res.mean_exec_time_ns})")
        kernel.last_exec_time_ns = res.exec_time_ns
    out = np.concatenate([res.results[c]["out"] for c in range(NCORES)],
                         axis=0)
    return out


# revision 46
# speedup vs baseline: 1.0869x; 1.0869x over previous
"""DisplaceChannel kernel for Trainium2 (8 NeuronCores, Bass/Tile).

out = depthwise3x3(displace(inp, round(offset)), gaussian(offset - round(offset)))

Strategy (v5):
- Data-parallel over batch: 32 batches -> 4 per core.
- Positions packed 4 per tile (4 pos x 4 batch x 8 chan = 128 partitions),
  grouped by EQUAL integer x-offset (sorted by y-offset inside a group so
  the row-band union stays tight).
- y-displacement folded into the input DMA row placement; rows copied
  FULL-WIDTH so each channel transfer is one contiguous Hv*256B chunk.
- x-displacement folded into the x-conv access-pattern offsets (uniform
  within a group). Data sits in unshifted "u" coords until the x-conv
  writes shifted output coords.
- Separable 3-tap y-conv then 3-tap x-conv, band/window-restricted.
  Each (group, stage) runs either on ScalarE+VectorE (per-partition-scalar
  mul + two fused scalar_tensor_tensor MACs) or on the TensorEngine
  (3 accumulating diagonal-stationary fp32 matmuls into PSUM + ScalarE
  copy back to SBUF). A greedy balancer splits stages across PE and DVE.
- Output HBM is pre-zeroed by the runtime; each position's nonzero row
  band is written full-width (contiguous), zeros in off-band columns.
"""
import os
import sys

import numpy as np

for _p in ("/opt/trn_rl_repo", "/root/.axon_site/_ro/trn_rl_repo"):
    if os.path.isdir(_p) and _p not in sys.path:
        sys.path.insert(0, _p)
        break

from contextlib import ExitStack

import concourse.bass as bass  # noqa: F401
import concourse.tile as tile
from concourse import bacc, mybir
from concourse.bass_utils import run_bass_kernel_spmd

H = 64
W = 64
B = 32
CHAN_PER_POS = 8
NUM_POS = 48
C = NUM_POS * CHAN_PER_POS
SIGMA = 0.5
NCORES = 8
BL = B // NCORES
POS_PER_GROUP = 4
F32 = mybir.dt.float32

# engine cost weights (ns per output column) for the stage balancer
_PE_NS_PER_COL = 5.0    # 3 x 4 cyc/row @ 2.4 GHz, measured
_PE_NS_PER_CHUNK = 1270.0  # 3 x (ldweights + matmul fixed), measured
_DVE_NS_PER_COL = 2.08  # 2 stt ops @ 0.96 GHz, measured
_DVE_NS_PER_STAGE = 310.0


def _pe_cost(cols):
    import math
    if cols < 2800:
        # narrow stages are per-matmul-overhead dominated on PE; keep on DVE
        return float("inf")
    return cols * _PE_NS_PER_COL + math.ceil(cols / 512) * _PE_NS_PER_CHUNK


def _dve_cost(cols):
    return cols * _DVE_NS_PER_COL + _DVE_NS_PER_STAGE

_cache = {}


def _geometry(offset):
    off_round = np.round(offset)  # round-half-even, matches jnp.round
    oxy = off_round.astype(np.int64)
    frac = (offset - off_round).astype(np.float32)

    coords = (np.arange(3, dtype=np.float32) - np.float32(1.0))
    dx = coords[None, :] + frac[:, 0:1]
    dy = coords[None, :] + frac[:, 1:2]
    inv = np.float32(1.0 / (2.0 * SIGMA * SIGMA))
    gx = np.exp(-(dx * dx) * inv).astype(np.float32)
    gy = np.exp(-(dy * dy) * inv).astype(np.float32)
    wx = gx / gx.sum(axis=1, keepdims=True)
    wy = gy / gy.sum(axis=1, keepdims=True)

    pos = {}
    for p in range(NUM_POS):
        ox, oy = int(oxy[p, 0]), int(oxy[p, 1])
        vy0, vy1 = max(0, oy), min(H, H + oy)
        vx0, vx1 = max(0, ox), min(W, W + ox)
        if vy1 <= vy0 or vx1 <= vx0:
            continue
        pos[p] = dict(
            p=p, ox=ox, oy=oy, vy0=vy0, vy1=vy1,
            sy0=vy0 - oy, sx0=vx0 - ox, wv=vx1 - vx0,
            by0=max(0, vy0 - 1), by1=min(H, vy1 + 1),
            bx0=max(0, vx0 - 1), bx1=min(W, vx1 + 1),
        )

    by_ox = {}
    for p, m in sorted(pos.items(), key=lambda kv: (kv[1]["ox"], kv[1]["oy"])):
        by_ox.setdefault(m["ox"], []).append(m)

    groups = []
    for ox in sorted(by_ox):
        # members are row-band-ALIGNED inside the tile (each band placed at
        # local row 0), so a group costs max(band), not the absolute union.
        # Group the biggest bands together to minimize sum-of-max.
        mem = sorted(by_ox[ox], key=lambda m: m["by0"] - m["by1"])
        for i in range(0, len(mem), POS_PER_GROUP):
            members = mem[i:i + POS_PER_GROUP]
            bg = max(m["by1"] - m["by0"] for m in members)
            sx0 = members[0]["sx0"]
            wv = members[0]["wv"]
            ud0 = max(0, sx0 - 2)
            ud1 = min(W, sx0 + wv + 2)
            groups.append(dict(
                members=members, ox=ox, bg=bg,
                sx0=sx0, wv=wv, ud0=ud0, ud1=ud1,
                bx0=members[0]["bx0"], bx1=members[0]["bx1"],
            ))

    # greedy PE/DVE balance over (group, stage) units
    units = []
    for g, grp in enumerate(groups):
        bg = grp["bg"]
        units.append((bg * (grp["ud1"] - grp["ud0"]), g, "y"))
        units.append((bg * (grp["bx1"] - grp["bx0"]), g, "x"))
    # walk in pipeline order so PE/DVE stages interleave in time
    units.sort(key=lambda u: (u[1], u[2] == "x"))
    pe_load, dve_load = 0.0, 0.0
    assign = {}
    use_pe = os.environ.get("KERNEL_USE_PE", "") == "1"
    for cols, g, st in units:
        if use_pe and max(pe_load + _pe_cost(cols), dve_load) <= \
           max(pe_load, dve_load + _dve_cost(cols)):
            assign[(g, st)] = "pe"
            pe_load += _pe_cost(cols)
        else:
            assign[(g, st)] = "dve"
            dve_load += _dve_cost(cols)
    for g, grp in enumerate(groups):
        grp["eng_y"] = assign[(g, "y")]
        grp["eng_x"] = assign[(g, "x")]

    ng = len(groups)
    taps = np.zeros((128, max(ng, 1) * 6), dtype=np.float32)
    for g, grp in enumerate(groups):
        for i, m in enumerate(grp["members"]):
            rows = slice(i * 32, (i + 1) * 32)
            for k in range(3):
                taps[rows, g * 6 + k] = wy[m["p"], k]
                taps[rows, g * 6 + 3 + k] = wx[m["p"], k]

    # diagonal stationaries for PE-assigned stages: [128, nd*128]
    diag_cols = []
    for g, grp in enumerate(groups):
        for st, wmat in (("y", wy), ("x", wx)):
            if grp["eng_" + st] != "pe":
                continue
            grp["diag_" + st] = len(diag_cols)
            for k in range(3):
                dcol = np.zeros((128, 128), dtype=np.float32)
                for i, m in enumerate(grp["members"]):
                    for q in range(i * 32, (i + 1) * 32):
                        dcol[q, q] = wmat[m["p"], k]
                diag_cols.append(dcol)
    diags = (np.concatenate(diag_cols, axis=1) if diag_cols
             else np.zeros((128, 128), dtype=np.float32))
    return groups, taps, diags


def _build(groups, n_tap_cols, n_diag_cols):
    nc = bacc.Bacc("TRN2", target_bir_lowering=False, debug=False,
                   num_devices=NCORES)
    inp_d = nc.dram_tensor("inp", [BL, C, H, W], F32, kind="ExternalInput")
    taps_d = nc.dram_tensor("taps", [128, n_tap_cols], F32, kind="ExternalInput")
    diags_d = nc.dram_tensor("diags", [128, n_diag_cols], F32,
                             kind="ExternalInput")
    out_d = nc.dram_tensor("out", [BL, C, H, W], F32, kind="ExternalOutput")

    mult = mybir.AluOpType.mult
    add = mybir.AluOpType.add
    dma_ctr = [0]

    with tile.TileContext(nc) as tc:
        with ExitStack() as ctx:
            use_pe_b = os.environ.get("KERNEL_USE_PE", "") == "1"
            dpool = ctx.enter_context(tc.tile_pool(name="dpool", bufs=4))
            tpool = ctx.enter_context(
                tc.tile_pool(name="tpool", bufs=3 if use_pe_b else 4))
            opool = ctx.enter_context(tc.tile_pool(name="opool", bufs=3))
            cpool = ctx.enter_context(tc.tile_pool(name="cpool", bufs=1))
            pspool = ctx.enter_context(
                tc.tile_pool(name="pspool", bufs=8, space="PSUM"))

            taps_t = cpool.tile([128, n_tap_cols], F32, tag="taps")
            nc.sync.dma_start(taps_t[:], taps_d.ap()[:, :])
            diags_t = cpool.tile([128, n_diag_cols], F32, tag="diags")
            nc.sync.dma_start(diags_t[:], diags_d.ap()[:, :])

            def tap(g, k):
                return taps_t[:, g * 6 + k:g * 6 + k + 1]

            def dma(dst, src):
                if os.environ.get("KERNEL_DMA_SYNC_ONLY", "") == "1":
                    eng = nc.sync
                else:
                    eng = (nc.sync, nc.scalar)[dma_ctr[0] % 2]
                dma_ctr[0] += 1
                eng.dma_start(dst, src)

            def stage_dve(out_ap, in_aps, tapbase, g):
                # split by row-halves: the stt MACs on the first half can
                # start while ScalarE is still doing the second half's mul
                rows = out_ap.shape[1]
                splits = ((0, rows),)
                if os.environ.get("KERNEL_ROW_SPLIT", "1") == "1" and rows >= 8:
                    h = rows // 2
                    splits = ((0, h), (h, rows))
                for (ra, rb) in splits:
                    nc.scalar.mul(out_ap[:, ra:rb], in_aps[0][:, ra:rb],
                                  tap(g, tapbase))
                for (ra, rb) in splits:
                    nc.vector.scalar_tensor_tensor(
                        out_ap[:, ra:rb], in_aps[1][:, ra:rb],
                        tap(g, tapbase + 1), out_ap[:, ra:rb], mult, add)
                    nc.vector.scalar_tensor_tensor(
                        out_ap[:, ra:rb], in_aps[2][:, ra:rb],
                        tap(g, tapbase + 2), out_ap[:, ra:rb], mult, add)

            def stage_pe(out3, ocol0, in3, icol0, wcols, bg, rows_all,
                         diag_idx):
                # out3[:, r, ocol0:ocol0+wcols] =
                #   sum_k diag_k * in3[:, r+dr_k, icol0:icol0+wcols]
                rpc = max(1, 512 // wcols)
                nchunks = -(-bg // rpc)
                rpc = -(-bg // nchunks)  # equal-ish chunks, no tiny tail
                r = 0
                while r < bg:
                    nr = min(rpc, bg - r)
                    acc = pspool.tile([128, nr * wcols], F32, tag="ps")
                    accv = acc[:].rearrange("q (a b) -> q a b", b=wcols)
                    for k in range(3):
                        dr = k if rows_all else 0
                        dc = 0 if rows_all else k
                        nc.tensor.matmul(
                            acc[:, 0:nr * wcols],
                            diags_t[:, (diag_idx + k) * 128:
                                    (diag_idx + k + 1) * 128],
                            in3[:, r + dr:r + dr + nr,
                                icol0 + dc:icol0 + dc + wcols],
                            start=(k == 0), stop=(k == 2))
                    nc.scalar.copy(out3[:, r:r + nr, ocol0:ocol0 + wcols],
                                   accv[:, :, :])
                    r += nr

            for g, grp in enumerate(groups):
                bg = grp["bg"]
                drows = bg + 2
                ox = grp["ox"]
                sx0, wv, ud0, ud1 = grp["sx0"], grp["wv"], grp["ud0"], grp["ud1"]
                wd = ud1 - ud0
                wt = wd + 4
                bx0, bx1 = grp["bx0"], grp["bx1"]
                wb = bx1 - bx0

                d_t = dpool.tile([128, drows * W], F32, tag="D")
                d3 = d_t[:].rearrange("q (r c) -> q r c", c=W)
                nc.gpsimd.memset(d3[:, :, ud0:ud1], 0.0)

                # partition layout within a member: q = i*32 + ch*4 + b, so
                # the DMA can put the 8-value channel dim outermost (the SDMA
                # engine index follows the outermost AP dim -> 8 engines)
                for i, m in enumerate(grp["members"]):
                    hv = m["vy1"] - m["vy0"]
                    r0 = 1 + m["vy0"] - m["by0"]
                    q0 = i * 32
                    dst = d_t[q0:q0 + 32, r0 * W:(r0 + hv) * W]
                    src = inp_d.ap()[:, 8 * m["p"]:8 * m["p"] + 8,
                                     m["sy0"]:m["sy0"] + hv, :]
                    dma(dst, src.rearrange("b ch r c -> ch b (r c)"))

                if sx0 > ud0:
                    nc.gpsimd.memset(d3[:, :, ud0:sx0], 0.0)
                if ud1 > sx0 + wv:
                    nc.gpsimd.memset(d3[:, :, sx0 + wv:ud1], 0.0)

                # y-conv: T[tr, 2+j] = sum_ky wy[ky] * D[tr+ky, ud0+j]
                t_t = tpool.tile([128, bg * wt], F32, tag="T")
                t3 = t_t[:].rearrange("q (r c) -> q r c", c=wt)
                nc.gpsimd.memset(t3[:, :, 0:2], 0.0)
                nc.gpsimd.memset(t3[:, :, wt - 2:wt], 0.0)
                if grp["eng_y"] == "pe":
                    stage_pe(t3, 2, d3, ud0, wd, bg, True, grp["diag_y"])
                else:
                    tdat = t3[:, :, 2:2 + wd]
                    stage_dve(tdat,
                              [d3[:, k:k + bg, ud0:ud1] for k in range(3)],
                              0, g)

                # x-conv: O[tr, x] = sum_kx wx[kx] * T[tr, x-ox+kx-1-ud0+2]
                o_t = opool.tile([128, bg * W], F32, tag="O")
                o3 = o_t[:].rearrange("q (r c) -> q r c", c=W)
                if bx0 > 0:
                    nc.gpsimd.memset(o3[:, :, 0:bx0], 0.0)
                if bx1 < W:
                    nc.gpsimd.memset(o3[:, :, bx1:W], 0.0)
                c0 = bx0 - ox - 1 - ud0 + 2
                if grp["eng_x"] == "pe":
                    stage_pe(o3, bx0, t3, c0, wb, bg, False, grp["diag_x"])
                else:
                    odat = o3[:, :, bx0:bx1]
                    stage_dve(odat,
                              [t3[:, :, c0 + k:c0 + k + wb] for k in range(3)],
                              3, g)

                for i, m in enumerate(grp["members"]):
                    r0, r1 = 0, m["by1"] - m["by0"]
                    q0 = i * 32
                    src = o_t[q0:q0 + 32, r0 * W:r1 * W]
                    dst = out_d.ap()[:, 8 * m["p"]:8 * m["p"] + 8,
                                     m["by0"]:m["by1"], :]
                    dma(dst.rearrange("b ch r c -> ch b (r c)"), src)

    nc.compile()
    return nc


def kernel(inp, offset):
    inp = np.ascontiguousarray(inp, dtype=np.float32)
    offset = np.ascontiguousarray(offset, dtype=np.float32)
    assert inp.shape == (B, C, H, W), inp.shape

    key = offset.tobytes()
    if key not in _cache:
        groups, taps, diags = _geometry(offset)
        nc = _build(groups, taps.shape[1], diags.shape[1])
        _cache[key] = (nc, taps, diags)
    nc, taps, diags = _cache[key]

    in_maps = [{"inp": inp[c * BL:(c + 1) * BL], "taps": taps, "diags": diags}
               for c in range(NCORES)]
    trace = os.environ.get("KERNEL_TRACE", "") == "1"
    try:
        res = run_bass_kernel_spmd(nc, in_maps, core_ids=list(range(NCORES)),
                                   trace=trace)
    except ModuleNotFoundError:
        # NTFF profile hook unavailable; run untraced
        trace = False
        res = run_bass_kernel_spmd(nc, in_maps, core_ids=list(range(NCORES)),
                                   trace=False)
    if trace:
        print(f"HW exec time: {res.exec_time_ns} ns "
              f"(mean {res.mean_exec_time_ns})")
        kernel.last_exec_time_ns = res.exec_time_ns
    out = np.concatenate([res.results[c]["out"] for c in range(NCORES)],
                         axis=0)
    return out
